# revision 34
# baseline (speedup 1.0000x reference)
"""Trainium2 Bass kernel: multi-head self-attention (B=4, N=2048, C=1024, H=16, D=64).

Sharding (zero-collective): core i = 2*b + hs handles batch b and head-half hs
(8 of 16 heads). Each core computes q/k/v for its 8 heads over all 2048
tokens, attention in the S^T orientation, and a PARTIAL output projection
(contraction over its 512 head-channels). The host adds the two partials per
batch — the "all-reduce after proj" is a free host-side add.

Schedule: 8 braid units = (head-pair p, q-half qh), ordered all-A then all-B
so the A-half projection runs as a mid-kernel burst. Unit u does S^T + exp
for its pair while the PV matmuls of unit u-1 interleave per key-tile jk.

PE p-state: any idle gap drops the clock 2.4->1.2 GHz and it takes 3us of
continuous work to ramp back, so PE density dominates everything. Per braid
slot the PE produces two [128,1024] score tiles (1727ns); consuming both on
ScalarE (2x1286ns) starves the PE, so each slot splits its two heads across
engines: h2=0 exp on ScalarE (1286ns), h2=1 on the DVE (1445ns) via a
Schraudolph bit-trick in bf16 space: i16 = x*(scale*log2e*128) + (127*128+c)
written as int16 and bitcast to bf16 gives 2^(x*log2e) with ~1.8% rms
sawtooth error (~1% end-to-end at 50% coverage; gate is 2e-2). The softmax
denominator sums the STORED weights (ones-column PV trick), so approximated
weights still normalize to exactly 1.

GEMM bursts (kt/qt of later pairs, projection) sit between units, where the
PV psum banks are free; burst chains stagger between the "at" and "pv" psum
rings so the first chains never wait on normalization reads.
"""

import numpy as np
import ml_dtypes

P = 128
C = 1024          # hidden
T = 2048          # tokens (q and kv)
HC = 8            # heads per core
D = 64            # head dim
KSUB = C // P     # 8 contraction subtiles
JK = T // P       # 16 key tiles
NPAIR = HC // 2   # 4 head pairs
TQ = 1024         # q tokens per braid unit (half of T)
SCALE = D ** -0.5

# Schraudolph fast-exp in bf16 bit space (see module docstring)
SCHRA_A = SCALE * np.log2(np.e) * 128.0
SCHRA_B = 127.0 * 128.0 - 7.0

BF16 = ml_dtypes.bfloat16

_CACHE = {}


def _build_nc():
    import concourse.bass as bass
    import concourse.bacc as bacc
    import concourse.mybir as mybir
    from concourse.bass import ds, ts
    from concourse.tile import TileContext
    from contextlib import ExitStack

    f32, bf16 = mybir.dt.float32, mybir.dt.bfloat16
    i16 = mybir.dt.int16
    AF = mybir.ActivationFunctionType
    OP = mybir.AluOpType

    import bass_rust as _bass_rust
    from concourse.hw_specs import get_activation_tables

    class _Bacc(bacc.Bacc):
        # Exp and Ln both live in natural_log_exp_and_others; restrict the
        # selector so it never thrashes between table sets.
        def insert_act_table_loads(self):
            has_activation = any(
                isinstance(i, mybir.InstActivation)
                for b in self.main_func.blocks
                for i in b.instructions
            )
            if not has_activation:
                return
            tables = []
            for k, v in get_activation_tables(self.m.arch).items():
                if k != "natural_log_exp_and_others":
                    v = frozenset(
                        f for f in v
                        if f not in (mybir.ActivationFunctionType.Exp,
                                     mybir.ActivationFunctionType.Ln))
                tables.append((k, v))
            _bass_rust.insert_act_table_loads(self, tables)

    nc = _Bacc()
    xbt_d = nc.dram_tensor("xbt", [C, T], bf16, kind="ExternalInput")
    wq_d = nc.dram_tensor("wq", [C, 4 * P], bf16, kind="ExternalInput")
    wk_d = nc.dram_tensor("wk", [C, 4 * P], bf16, kind="ExternalInput")
    wv_d = nc.dram_tensor("wv", [C, 4 * P], bf16, kind="ExternalInput")
    wp_d = nc.dram_tensor("wp", [4 * P, C], bf16, kind="ExternalInput")
    bq_d = nc.dram_tensor("bq", [4 * P], f32, kind="ExternalInput")
    bk_d = nc.dram_tensor("bk", [4 * P], f32, kind="ExternalInput")
    bv_d = nc.dram_tensor("bv", [4 * P], f32, kind="ExternalInput")
    out_d = nc.dram_tensor("out", [T, C], bf16, kind="ExternalOutput")

    wq_r = wq_d.rearrange("(o p) n -> p o n", p=P)   # [128, 8, 512]
    wk_r = wk_d.rearrange("(o p) n -> p o n", p=P)
    wv_r = wv_d.rearrange("(o p) n -> p o n", p=P)
    wp_r = wp_d.rearrange("(o p) n -> p o n", p=P)   # [128, 4, 1024]

    def bcast_ap(row_d):
        # DRAM vector replicated across all 128 partitions via step-0 AP
        row = row_d[:]
        return bass.AP(tensor=row.tensor, offset=row.offset,
                       ap=[[0, P], *row.ap])

    with ExitStack() as ctx:
        tc = ctx.enter_context(TileContext(nc))
        singles = ctx.enter_context(tc.tile_pool(name="singles", bufs=1))
        psum = ctx.enter_context(tc.tile_pool(name="psum", bufs=2, space="PSUM"))
        wpool = ctx.enter_context(tc.tile_pool(name="wpool", bufs=1))
        kvq = ctx.enter_context(tc.tile_pool(name="kvq", bufs=1))
        ptp = ctx.enter_context(tc.tile_pool(name="ptp", bufs=1))
        spool = ctx.enter_context(tc.tile_pool(name="spool", bufs=1))
        ypool = ctx.enter_context(tc.tile_pool(name="ypool", bufs=1))

        # ---- v bias (gpsimd queue, parallel with the sync-queue loads) ----
        vbias = singles.tile([P, 4 * P], f32)
        nc.gpsimd.dma_start(vbias, bcast_ap(bv_d))
        vbias_h = vbias.rearrange("p (h c) -> p h c", c=D)

        # ---- pair-0 qkv weights first: they gate the first matmul ----
        w0 = {}
        for nm, r in (("wk", wk_r), ("wq", wq_r)):
            wt = wpool.tile([P, KSUB, P], bf16, tag="wsm", bufs=2,
                            name=f"{nm}0")
            nc.sync.dma_start(wt, r[:, :, ts(0, P)])
            w0[nm] = wt

        # ---- x^T ----
        xbt = singles.tile([P, KSUB, T], bf16)
        xbt_r = xbt_d.rearrange("(o p) t -> p o t", p=P)
        for k in range(KSUB):
            nc.sync.dma_start(xbt[:, k:k + 1, :], xbt_r[:, k:k + 1, :])

        # ---- q/k bias columns (tiny; needed only after the first chains) ----
        bqc = singles.tile([P, NPAIR], f32)
        nc.sync.dma_start(bqc, bq_d.rearrange("(o p) -> p o", p=P))
        bkc = singles.tile([P, NPAIR], f32)
        nc.sync.dma_start(bkc, bk_d.rearrange("(o p) -> p o", p=P))

        # ---- V weights (used in unit 0); wp shares the tag, loaded after ----
        wv_t = singles.tile([P, KSUB, 4 * P], bf16, tag="wbig", bufs=1,
                            name="wv")
        nc.sync.dma_start(wv_t, wv_r)
        wp_holder = {}

        # ---- V_aug tiles: [128 tokens, 8 heads, 96] (cols 64-95 = ones) ----
        ONE = 64
        va_tiles = []
        for jk in range(JK):
            va = kvq.tile([P, HC, D + ONE], bf16, tag=f"va{jk}", bufs=1,
                          name=f"va{jk}")
            nc.gpsimd.memset(va[:, :, D:D + ONE], 1.0)
            va_tiles.append(va)

        kt_tiles = [None] * NPAIR
        qt_tiles = [None] * NPAIR

        # psum staging for GEMM-burst chains: alternate between the "at" and
        # "pv" rings so the first chains of a burst reuse "at" slots (already
        # freed by the last braid exps) instead of waiting on the norm reads
        # that release the "pv" slots.
        burst_seq = {"i": 0}

        def burst_ps(name):
            i = burst_seq["i"]
            burst_seq["i"] += 1
            if i % 4 < 2:
                return psum.tile([P, TQ], f32, tag="at", bufs=2, name=name)[:, 0:512]
            return psum.tile([P, 512], f32, tag="pv", bufs=4, name=name)

        def emit_kt_qt(p):
            # K^T / Q^T for pair p: [128 cols (heads 2p,2p+1), 2048 tokens]
            if p == 0:
                wkp, wqp = w0["wk"], w0["wq"]
            else:
                wkp = wpool.tile([P, KSUB, P], bf16, tag="wsm", bufs=2,
                                 name=f"wk{p}")
                nc.sync.dma_start(wkp, wk_r[:, :, ts(p, P)])
                wqp = wpool.tile([P, KSUB, P], bf16, tag="wsm", bufs=2,
                                 name=f"wq{p}")
                nc.sync.dma_start(wqp, wq_r[:, :, ts(p, P)])
            kt = kvq.tile([P, T], bf16, tag=f"kt{p}", bufs=1, name=f"kt{p}")
            qt = kvq.tile([P, T], bf16, tag=f"qt{p}", bufs=1, name=f"qt{p}")
            for quarter in range(4):
                ps = burst_ps(f"ktps{p}_{quarter}")
                for k in range(KSUB):
                    nc.tensor.matmul(
                        ps, wkp[:, k, :], xbt[:, k, ts(quarter, 512)],
                        start=(k == 0), stop=(k == KSUB - 1),
                    )
                nc.scalar.activation(kt[:, ts(quarter, 512)], ps, AF.Identity,
                                     bias=bkc[:, p:p + 1])
                ps = burst_ps(f"qtps{p}_{quarter}")
                for k in range(KSUB):
                    nc.tensor.matmul(
                        ps, wqp[:, k, :], xbt[:, k, ts(quarter, 512)],
                        start=(k == 0), stop=(k == KSUB - 1),
                    )
                nc.scalar.activation(qt[:, ts(quarter, 512)], ps, AF.Identity,
                                     bias=bqc[:, p:p + 1])
            kt_tiles[p] = kt
            qt_tiles[p] = qt

        def emit_v_slot(t2):
            # V for all 8 heads, token tile t2: [128 tokens, 512 vcols]
            ps = psum.tile([P, 512], f32, tag="pv", bufs=4, name=f"vps{t2}")
            for k in range(KSUB):
                nc.tensor.matmul(
                    ps, xbt[:, k, ts(t2, P)], wv_t[:, k, :],
                    start=(k == 0), stop=(k == KSUB - 1),
                )
            nc.vector.tensor_tensor(
                va_tiles[t2][:, :, 0:D],
                ps.rearrange("p (e c) -> p e c", c=D),
                vbias_h,
                OP.add,
            )

        # ---- attention: fine-braided units ----
        UNITS = [(p, 0) for p in range(NPAIR)] + [(p, 1) for p in range(NPAIR)]
        obuf_tiles = {}  # (p, qh) -> tile
        pt_tiles = {}    # (jk, h2) -> tile (ring of 1 per tag)
        unit_pv = {}     # u -> {(h2, ic): psum tile}

        def exp_on_dve(u, jk, h2, naked):
            if u == 0:
                return h2 == 1
            if jk in (0, 1):
                return True          # entry shield: ScalarE drains the lump
            if naked:
                if jk in (7, 10, 12):
                    return False     # repay the DVE mid-unit
                return h2 == 1 or jk == JK - 1
            if jk in (7, 10):
                return False
            return h2 == 1

        def emit_st_group(u, jk, h2, naked=False):
            p, qh = UNITS[u]
            kt, qt = kt_tiles[p], qt_tiles[p]
            ps = psum.tile([P, TQ], f32, tag="at", bufs=2,
                           name=f"stps{u}_{jk}_{h2}")
            for ic in range(2):
                nc.tensor.matmul(
                    ps[:, ts(ic, 512)],
                    kt[ds(h2 * D, D), ts(jk, P)],
                    qt[ds(h2 * D, D), ds(qh * TQ + ic * 512, 512)],
                    start=True, stop=True,
                )
            pt = ptp.tile([P, TQ], bf16, tag=f"pt{jk}_{h2}", bufs=1,
                          name=f"pt{u}_{jk}_{h2}")
            if exp_on_dve(u, jk, h2, naked):
                nc.vector.tensor_scalar(
                    pt.bitcast(i16), ps, SCHRA_A, SCHRA_B, OP.mult, OP.add)
            else:
                nc.scalar.activation(pt, ps, AF.Exp, scale=SCALE)
            pt_tiles[(jk, h2)] = pt

        def emit_pv_slot(u, jk):
            # one jk step of the 4 PV chains of unit u
            p, qh = UNITS[u]
            if u not in unit_pv:
                unit_pv[u] = {
                    (h2, ic): psum.tile([D + ONE, 512], f32, tag="pv", bufs=4,
                                        name=f"pv{u}_{h2}_{ic}")
                    for h2 in range(2) for ic in range(2)
                }
            for h2 in range(2):
                for ic in range(2):
                    nc.tensor.matmul(
                        unit_pv[u][(h2, ic)],
                        va_tiles[jk][:, 2 * p + h2, :],
                        pt_tiles[(jk, h2)][:, ts(ic, 512)],
                        start=(jk == 0), stop=(jk == JK - 1),
                    )

        def get_obuf(u):
            p, qh = UNITS[u]
            if (p, qh) not in obuf_tiles:
                obuf_tiles[(p, qh)] = kvq.tile(
                    [P, TQ], bf16, tag=f"ob{p}_{qh}", bufs=1,
                    name=f"ob{p}_{qh}")
            return obuf_tiles[(p, qh)]

        def emit_norm_set(u, h2, ic):
            # obuf rows for head h2 = pv rows 0-63 scaled by 1/denominator
            # (rows 64-127, replicated by the ones block)
            p, qh = UNITS[u]
            obuf = get_obuf(u)
            pv = unit_pv[u][(h2, ic)]
            rs = spool.tile([ONE, 512], f32, tag="rs", bufs=2,
                            name=f"rs{u}_{h2}_{ic}")
            nc.scalar.activation(rs, pv[ds(D, ONE), :], AF.Ln)
            nc.scalar.activation(rs, rs, AF.Exp, scale=-1.0)
            nc.vector.tensor_tensor(
                obuf[ds(h2 * D, D), ts(ic, 512)],
                pv[ds(0, D), :],
                rs[:, :],
                OP.mult,
            )

        def emit_norm(u):
            for h2 in range(2):
                for ic in range(2):
                    emit_norm_set(u, h2, ic)

        # units NOT followed by a PE-only GEMM burst: the norm lump there
        # must not delay ScalarE's last exp (it gates the "at" psum ring),
        # so slot 15 sends both heads to the DVE and norms after.
        NAKED = (3, 4, 6, 7)

        def emit_unit(u):
            prev = u - 1
            for jk in range(JK):
                if prev >= 0:
                    emit_pv_slot(prev, jk)
                    if jk == JK - 1 and u not in NAKED:
                        emit_norm(prev)
                for h2 in range(2):
                    emit_st_group(u, jk, h2, naked=(u in NAKED))
                if u == 0:
                    emit_v_slot(jk)
            if prev >= 0 and u in NAKED:
                emit_norm(prev)

        def emit_proj(qh, its=None):
            # one [128,1024] psum per token chunk: two 4-matmul chains into
            # its halves, one wide ScalarE copy, one DMA — fewer ring
            # rotations so the chains never leapfrog-stall on the copies
            for it in (range(TQ // P) if its is None else its):
                ps = psum.tile([P, TQ], f32, tag="at", bufs=2,
                               name=f"yps{qh}_{it}")
                for n in range(2):
                    for mm in range(NPAIR):
                        nc.tensor.matmul(
                            ps[:, ts(n, 512)],
                            obuf_tiles[(mm, qh)][:, ts(it, P)],
                            wp_holder["wp"][:, mm, ts(n, 512)],
                            start=(mm == 0), stop=(mm == NPAIR - 1),
                        )
                y = ypool.tile([P, TQ], bf16, tag="y", bufs=2,
                               name=f"y{qh}_{it}")
                nc.scalar.activation(y, ps, AF.Copy)
                nc.sync.dma_start(out_d[ds(qh * TQ + it * P, P), :], y)

        emit_kt_qt(0)
        for u in range(len(UNITS)):
            emit_unit(u)
            if u == 0:
                wp_t = singles.tile([P, NPAIR, C], bf16, tag="wbig", bufs=1,
                                    name="wp")
                nc.sync.dma_start(wp_t, wp_r)
                wp_holder["wp"] = wp_t
            if u <= 2:
                emit_kt_qt(u + 1)
            elif u == 5:
                emit_proj(0)

        # tail: sequential PV chains for the last unit, each normed as it
        # stops, so the B-half projection never waits on a fresh norm
        last = len(UNITS) - 1
        p_last, _ = UNITS[last]
        unit_pv[last] = {}
        for idx, (h2, ic) in enumerate(((0, 0), (1, 0), (0, 1), (1, 1))):
            pv = psum.tile([D + ONE, 512], f32, tag="pv", bufs=4,
                           name=f"pv{last}_{h2}_{ic}")
            unit_pv[last][(h2, ic)] = pv
            for jk in range(JK):
                nc.tensor.matmul(
                    pv,
                    va_tiles[jk][:, 2 * p_last + h2, :],
                    pt_tiles[(jk, h2)][:, ts(ic, 512)],
                    start=(jk == 0), stop=(jk == JK - 1),
                )
            emit_norm_set(last, h2, ic)
            if idx == 1:
                # ic=0 columns of the last obuf are final: overlap the first
                # half of the B projection with the remaining PV chains
                emit_proj(1, its=range(0, 4))
        emit_proj(1, its=range(4, 8))

    if not nc.is_finalized():
        nc.finalize()
    return nc


def get_nc():
    if "nc" not in _CACHE:
        _CACHE["nc"] = _build_nc()
    return _CACHE["nc"]


def make_in_maps(x, w_qkv, b_qkv, w_proj, b_proj):
    x = np.asarray(x)
    w_qkv = np.asarray(w_qkv)
    b_qkv = np.asarray(b_qkv, dtype=np.float32)
    w_proj = np.asarray(w_proj)
    b_proj = np.asarray(b_proj, dtype=np.float32)

    xbts = [np.ascontiguousarray(x[b].T).astype(BF16) for b in range(4)]

    in_maps = []
    for core in range(8):
        b, hs = divmod(core, 2)
        lo = hs * 512
        wq = np.ascontiguousarray(w_qkv[:, lo:lo + 512]).astype(BF16)
        wk = np.ascontiguousarray(w_qkv[:, C + lo:C + lo + 512]).astype(BF16)
        wv = np.ascontiguousarray(w_qkv[:, 2 * C + lo:2 * C + lo + 512]).astype(BF16)
        wp = np.ascontiguousarray(w_proj[lo:lo + 512, :]).astype(BF16)
        bq = np.ascontiguousarray(b_qkv[lo:lo + 512])
        bk = np.ascontiguousarray(b_qkv[C + lo:C + lo + 512])
        bv = np.ascontiguousarray(b_qkv[2 * C + lo:2 * C + lo + 512])
        in_maps.append(dict(xbt=xbts[b], wq=wq, wk=wk, wv=wv, wp=wp,
                            bq=bq, bk=bk, bv=bv))
    return in_maps


def run(x, w_qkv, b_qkv, w_proj, b_proj, trace=False, **kwargs):
    from concourse.bass_utils import run_bass_kernel_spmd
    nc = get_nc()
    in_maps = make_in_maps(x, w_qkv, b_qkv, w_proj, b_proj)
    res = run_bass_kernel_spmd(nc, in_maps, core_ids=list(range(8)),
                               trace=trace, **kwargs)
    B = 4
    bp = np.asarray(b_proj, dtype=np.float32)
    out = np.empty((B, T, C), np.float32)
    for b in range(B):
        out[b] = (res.results[2 * b]["out"].astype(np.float32)
                  + res.results[2 * b + 1]["out"].astype(np.float32)
                  + bp)
    return out, res


def kernel(x, w_qkv, b_qkv, w_proj, b_proj):
    out, _ = run(x, w_qkv, b_qkv, w_proj, b_proj, trace=False)
    return out


# revision 38
# speedup vs baseline: 1.0216x; 1.0216x over previous
"""Trainium2 Bass kernel: multi-head self-attention (B=4, N=2048, C=1024, H=16, D=64).

Sharding (zero-collective): core i = 2*b + hs handles batch b and head-half hs
(8 of 16 heads). Each core computes q/k/v for its 8 heads over all 2048
tokens, attention in the S^T orientation, and a PARTIAL output projection
(contraction over its 512 head-channels). The host adds the two partials per
batch — the "all-reduce after proj" is a free host-side add.

Schedule: 8 braid units = (head-pair p, q-half qh), ordered all-A then all-B
so the A-half projection runs as a mid-kernel burst. Unit u does S^T + exp
for its pair while the PV matmuls of unit u-1 interleave per key-tile jk.

PE p-state: any idle gap drops the clock 2.4->1.2 GHz and it takes 3us of
continuous work to ramp back, so PE density dominates everything. Per braid
slot the PE produces two [128,1024] score tiles (1727ns); consuming both on
ScalarE (2x1286ns) starves the PE, so each slot splits its two heads across
engines: h2=0 exp on ScalarE (1286ns), h2=1 on the DVE (1445ns) via a
Schraudolph bit-trick in bf16 space: i16 = x*(scale*log2e*128) + (127*128+c)
written as int16 and bitcast to bf16 gives 2^(x*log2e) with ~1.8% rms
sawtooth error (~1% end-to-end at 50% coverage; gate is 2e-2). The softmax
denominator sums the STORED weights (ones-column PV trick), so approximated
weights still normalize to exactly 1.

GEMM bursts (kt/qt of later pairs, projection) sit between units, where the
PV psum banks are free; burst chains stagger between the "at" and "pv" psum
rings so the first chains never wait on normalization reads.
"""

import numpy as np
import ml_dtypes

P = 128
C = 1024          # hidden
T = 2048          # tokens (q and kv)
HC = 8            # heads per core
D = 64            # head dim
KSUB = C // P     # 8 contraction subtiles
JK = T // P       # 16 key tiles
NPAIR = HC // 2   # 4 head pairs
TQ = 1024         # q tokens per braid unit (half of T)
SCALE = D ** -0.5

# Schraudolph fast-exp in bf16 bit space (see module docstring)
SCHRA_A = SCALE * np.log2(np.e) * 128.0
SCHRA_B = 127.0 * 128.0 - 7.0

BF16 = ml_dtypes.bfloat16

_CACHE = {}


def _build_nc():
    import concourse.bass as bass
    import concourse.bacc as bacc
    import concourse.mybir as mybir
    from concourse.bass import ds, ts
    from concourse.tile import TileContext
    from contextlib import ExitStack

    f32, bf16 = mybir.dt.float32, mybir.dt.bfloat16
    i16 = mybir.dt.int16
    AF = mybir.ActivationFunctionType
    OP = mybir.AluOpType

    import bass_rust as _bass_rust
    from concourse.hw_specs import get_activation_tables

    class _Bacc(bacc.Bacc):
        # Exp and Ln both live in natural_log_exp_and_others; restrict the
        # selector so it never thrashes between table sets.
        def insert_act_table_loads(self):
            has_activation = any(
                isinstance(i, mybir.InstActivation)
                for b in self.main_func.blocks
                for i in b.instructions
            )
            if not has_activation:
                return
            tables = []
            for k, v in get_activation_tables(self.m.arch).items():
                if k != "natural_log_exp_and_others":
                    v = frozenset(
                        f for f in v
                        if f not in (mybir.ActivationFunctionType.Exp,
                                     mybir.ActivationFunctionType.Ln))
                tables.append((k, v))
            _bass_rust.insert_act_table_loads(self, tables)

    nc = _Bacc()
    xbt_d = nc.dram_tensor("xbt", [C, T], bf16, kind="ExternalInput")
    wq_d = nc.dram_tensor("wq", [C, 4 * P], bf16, kind="ExternalInput")
    wk_d = nc.dram_tensor("wk", [C, 4 * P], bf16, kind="ExternalInput")
    wv_d = nc.dram_tensor("wv", [C, 4 * P], bf16, kind="ExternalInput")
    wp_d = nc.dram_tensor("wp", [4 * P, C], bf16, kind="ExternalInput")
    bq_d = nc.dram_tensor("bq", [4 * P], f32, kind="ExternalInput")
    bk_d = nc.dram_tensor("bk", [4 * P], f32, kind="ExternalInput")
    bv_d = nc.dram_tensor("bv", [4 * P], f32, kind="ExternalInput")
    out_d = nc.dram_tensor("out", [T, C], bf16, kind="ExternalOutput")

    wq_r = wq_d.rearrange("(o p) n -> p o n", p=P)   # [128, 8, 512]
    wk_r = wk_d.rearrange("(o p) n -> p o n", p=P)
    wv_r = wv_d.rearrange("(o p) n -> p o n", p=P)
    wp_r = wp_d.rearrange("(o p) n -> p o n", p=P)   # [128, 4, 1024]

    def bcast_ap(row_d):
        # DRAM vector replicated across all 128 partitions via step-0 AP
        row = row_d[:]
        return bass.AP(tensor=row.tensor, offset=row.offset,
                       ap=[[0, P], *row.ap])

    with ExitStack() as ctx:
        tc = ctx.enter_context(TileContext(nc))
        singles = ctx.enter_context(tc.tile_pool(name="singles", bufs=1))
        psum = ctx.enter_context(tc.tile_pool(name="psum", bufs=2, space="PSUM"))
        wpool = ctx.enter_context(tc.tile_pool(name="wpool", bufs=1))
        kvq = ctx.enter_context(tc.tile_pool(name="kvq", bufs=1))
        ptp = ctx.enter_context(tc.tile_pool(name="ptp", bufs=1))
        spool = ctx.enter_context(tc.tile_pool(name="spool", bufs=1))
        ypool = ctx.enter_context(tc.tile_pool(name="ypool", bufs=1))

        # ---- v bias (gpsimd queue, parallel with the sync-queue loads) ----
        vbias = singles.tile([P, 4 * P], f32)
        nc.gpsimd.dma_start(vbias, bcast_ap(bv_d))
        vbias_h = vbias.rearrange("p (h c) -> p h c", c=D)

        # ---- pair-0 qkv weights first: they gate the first matmul ----
        w0 = {}
        for nm, r in (("wk", wk_r), ("wq", wq_r)):
            wt = wpool.tile([P, KSUB, P], bf16, tag="wsm", bufs=2,
                            name=f"{nm}0")
            nc.sync.dma_start(wt, r[:, :, ts(0, P)])
            w0[nm] = wt

        # ---- x^T ----
        xbt = singles.tile([P, KSUB, T], bf16)
        xbt_r = xbt_d.rearrange("(o p) t -> p o t", p=P)
        for k in range(KSUB):
            nc.sync.dma_start(xbt[:, k:k + 1, :], xbt_r[:, k:k + 1, :])

        # ---- q/k bias columns (tiny; needed only after the first chains) ----
        bqc = singles.tile([P, NPAIR], f32)
        nc.sync.dma_start(bqc, bq_d.rearrange("(o p) -> p o", p=P))
        bkc = singles.tile([P, NPAIR], f32)
        nc.sync.dma_start(bkc, bk_d.rearrange("(o p) -> p o", p=P))

        # ---- V weights (used in unit 0); wp shares the tag, loaded after ----
        wv_t = singles.tile([P, KSUB, 4 * P], bf16, tag="wbig", bufs=1,
                            name="wv")
        nc.sync.dma_start(wv_t, wv_r)
        wp_holder = {}

        # ---- V_aug tiles: [128 tokens, 8 heads, 96] (cols 64-95 = ones) ----
        ONE = 64
        va_tiles = []
        for jk in range(JK):
            va = kvq.tile([P, HC, D + ONE], bf16, tag=f"va{jk}", bufs=1,
                          name=f"va{jk}")
            nc.gpsimd.memset(va[:, :, D:D + ONE], 1.0)
            va_tiles.append(va)

        kt_tiles = [None] * NPAIR
        qt_tiles = [None] * NPAIR

        # psum staging for GEMM-burst chains: alternate between the "at" and
        # "pv" rings so the first chains of a burst reuse "at" slots (already
        # freed by the last braid exps) instead of waiting on the norm reads
        # that release the "pv" slots.
        burst_seq = {"i": 0}

        def burst_ps(name):
            i = burst_seq["i"]
            burst_seq["i"] += 1
            if i % 4 < 2:
                return psum.tile([P, TQ], f32, tag="at", bufs=2, name=name)[:, 0:512]
            return psum.tile([P, 512], f32, tag="pv", bufs=4, name=name)

        def emit_kt_qt(p):
            # K^T / Q^T for pair p: [128 cols (heads 2p,2p+1), 2048 tokens]
            if p == 0:
                wkp, wqp = w0["wk"], w0["wq"]
            else:
                wkp = wpool.tile([P, KSUB, P], bf16, tag="wsm", bufs=2,
                                 name=f"wk{p}")
                nc.sync.dma_start(wkp, wk_r[:, :, ts(p, P)])
                wqp = wpool.tile([P, KSUB, P], bf16, tag="wsm", bufs=2,
                                 name=f"wq{p}")
                nc.sync.dma_start(wqp, wq_r[:, :, ts(p, P)])
            kt = kvq.tile([P, T], bf16, tag=f"kt{p}", bufs=1, name=f"kt{p}")
            qt = kvq.tile([P, T], bf16, tag=f"qt{p}", bufs=1, name=f"qt{p}")
            for quarter in range(4):
                ps = burst_ps(f"ktps{p}_{quarter}")
                for k in range(KSUB):
                    nc.tensor.matmul(
                        ps, wkp[:, k, :], xbt[:, k, ts(quarter, 512)],
                        start=(k == 0), stop=(k == KSUB - 1),
                    )
                nc.scalar.activation(kt[:, ts(quarter, 512)], ps, AF.Identity,
                                     bias=bkc[:, p:p + 1])
                ps = burst_ps(f"qtps{p}_{quarter}")
                for k in range(KSUB):
                    nc.tensor.matmul(
                        ps, wqp[:, k, :], xbt[:, k, ts(quarter, 512)],
                        start=(k == 0), stop=(k == KSUB - 1),
                    )
                nc.scalar.activation(qt[:, ts(quarter, 512)], ps, AF.Identity,
                                     bias=bqc[:, p:p + 1])
            kt_tiles[p] = kt
            qt_tiles[p] = qt

        def emit_v_slot(t2):
            # V for all 8 heads, token tile t2: [128 tokens, 512 vcols]
            ps = psum.tile([P, 512], f32, tag="pv", bufs=4, name=f"vps{t2}")
            for k in range(KSUB):
                nc.tensor.matmul(
                    ps, xbt[:, k, ts(t2, P)], wv_t[:, k, :],
                    start=(k == 0), stop=(k == KSUB - 1),
                )
            nc.vector.tensor_tensor(
                va_tiles[t2][:, :, 0:D],
                ps.rearrange("p (e c) -> p e c", c=D),
                vbias_h,
                OP.add,
            )

        # ---- attention: fine-braided units ----
        UNITS = [(p, 0) for p in range(NPAIR)] + [(p, 1) for p in range(NPAIR)]
        obuf_tiles = {}  # (p, qh) -> tile
        pt_tiles = {}    # (jk, h2) -> tile (ring of 1 per tag)
        unit_pv = {}     # u -> {(h2, ic): psum tile}

        def exp_on_dve(u, jk, h2, naked):
            if u == 0:
                return h2 == 1
            if jk in (0, 1):
                return True          # entry shield: ScalarE drains the lump
            if naked:
                if jk in (7, 10, 12):
                    return False     # repay the DVE mid-unit
                return h2 == 1 or jk == JK - 1
            if jk in (7, 10):
                return False
            return h2 == 1

        def emit_st_group(u, jk, h2, naked=False):
            p, qh = UNITS[u]
            kt, qt = kt_tiles[p], qt_tiles[p]
            ps = psum.tile([P, TQ], f32, tag="at", bufs=2,
                           name=f"stps{u}_{jk}_{h2}")
            for ic in range(2):
                nc.tensor.matmul(
                    ps[:, ts(ic, 512)],
                    kt[ds(h2 * D, D), ts(jk, P)],
                    qt[ds(h2 * D, D), ds(qh * TQ + ic * 512, 512)],
                    start=True, stop=True,
                )
            pt = ptp.tile([P, TQ], bf16, tag=f"pt{jk}_{h2}", bufs=1,
                          name=f"pt{u}_{jk}_{h2}")
            if exp_on_dve(u, jk, h2, naked):
                nc.vector.tensor_scalar(
                    pt.bitcast(i16), ps, SCHRA_A, SCHRA_B, OP.mult, OP.add)
            else:
                nc.scalar.activation(pt, ps, AF.Exp, scale=SCALE)
            pt_tiles[(jk, h2)] = pt

        def emit_pv_slot(u, jk):
            # one jk step of the 4 PV chains of unit u
            p, qh = UNITS[u]
            if u not in unit_pv:
                unit_pv[u] = {
                    (h2, ic): psum.tile([D + ONE, 512], f32, tag="pv", bufs=4,
                                        name=f"pv{u}_{h2}_{ic}")
                    for h2 in range(2) for ic in range(2)
                }
            for h2 in range(2):
                for ic in range(2):
                    nc.tensor.matmul(
                        unit_pv[u][(h2, ic)],
                        va_tiles[jk][:, 2 * p + h2, :],
                        pt_tiles[(jk, h2)][:, ts(ic, 512)],
                        start=(jk == 0), stop=(jk == JK - 1),
                    )

        def get_obuf(u):
            p, qh = UNITS[u]
            if (p, qh) not in obuf_tiles:
                obuf_tiles[(p, qh)] = kvq.tile(
                    [P, TQ], bf16, tag=f"ob{p}_{qh}", bufs=1,
                    name=f"ob{p}_{qh}")
            return obuf_tiles[(p, qh)]

        def emit_norm_set(u, h2, ic, dve_mult=False):
            # obuf rows for head h2 = pv rows 0-63 scaled by 1/denominator
            # (rows 64-127, replicated by the ones block). The multiply runs
            # on the otherwise-idle GpSimd: the DVE only stages the psum
            # values into SBUF (cheaper than the TT it replaces, and frees
            # the pv ring earlier); rs/staging rows are allocated at the
            # same partitions as obuf since GpSimd lanes cannot cross
            # partitions.
            p, qh = UNITS[u]
            obuf = get_obuf(u)
            pv = unit_pv[u][(h2, ic)]
            ro = ds(h2 * D, D)
            rs = spool.tile([P, 512], f32, tag="rs", bufs=2,
                            name=f"rs{u}_{h2}_{ic}")
            nc.scalar.activation(rs[ro, :], pv[ds(D, ONE), :], AF.Ln)
            nc.scalar.activation(rs[ro, :], rs[ro, :], AF.Exp, scale=-1.0)
            st = spool.tile([P, 512], bf16, tag="st", bufs=2,
                            name=f"st{u}_{h2}_{ic}")
            nc.vector.tensor_copy(st[ro, :], pv[ds(0, D), :])
            # mid-kernel norms have units of slack -> idle GpSimd; tail norms
            # gate the B-half projection immediately -> the (then-idle) DVE
            eng = nc.vector if dve_mult else nc.gpsimd
            eng.tensor_tensor(
                obuf[ro, ts(ic, 512)],
                st[ro, :],
                rs[ro, :],
                OP.mult,
            )

        def emit_norm(u):
            for h2 in range(2):
                for ic in range(2):
                    emit_norm_set(u, h2, ic)

        # units NOT followed by a PE-only GEMM burst: the norm lump there
        # must not delay ScalarE's last exp (it gates the "at" psum ring),
        # so slot 15 sends both heads to the DVE and norms after.
        NAKED = (3, 4, 6, 7)

        def emit_unit(u):
            prev = u - 1
            for jk in range(JK):
                if prev >= 0:
                    emit_pv_slot(prev, jk)
                    if jk == JK - 1 and u not in NAKED:
                        emit_norm(prev)
                for h2 in range(2):
                    emit_st_group(u, jk, h2, naked=(u in NAKED))
                if u == 0:
                    emit_v_slot(jk)
            if prev >= 0 and u in NAKED:
                emit_norm(prev)

        def emit_proj(qh, its=None):
            # one [128,1024] psum per token chunk: two 4-matmul chains into
            # its halves, one wide ScalarE copy, one DMA — fewer ring
            # rotations so the chains never leapfrog-stall on the copies
            for it in (range(TQ // P) if its is None else its):
                ps = psum.tile([P, TQ], f32, tag="at", bufs=2,
                               name=f"yps{qh}_{it}")
                for n in range(2):
                    for mm in range(NPAIR):
                        nc.tensor.matmul(
                            ps[:, ts(n, 512)],
                            obuf_tiles[(mm, qh)][:, ts(it, P)],
                            wp_holder["wp"][:, mm, ts(n, 512)],
                            start=(mm == 0), stop=(mm == NPAIR - 1),
                        )
                y = ypool.tile([P, TQ], bf16, tag="y", bufs=2,
                               name=f"y{qh}_{it}")
                nc.scalar.activation(y, ps, AF.Copy)
                nc.sync.dma_start(out_d[ds(qh * TQ + it * P, P), :], y)

        emit_kt_qt(0)
        for u in range(len(UNITS)):
            emit_unit(u)
            if u == 0:
                wp_t = singles.tile([P, NPAIR, C], bf16, tag="wbig", bufs=1,
                                    name="wp")
                nc.sync.dma_start(wp_t, wp_r)
                wp_holder["wp"] = wp_t
            if u <= 2:
                emit_kt_qt(u + 1)
            elif u == 5:
                emit_proj(0)

        # tail: sequential PV chains for the last unit, each normed as it
        # stops, so the B-half projection never waits on a fresh norm
        last = len(UNITS) - 1
        p_last, _ = UNITS[last]
        unit_pv[last] = {}
        for idx, (h2, ic) in enumerate(((0, 0), (1, 0), (0, 1), (1, 1))):
            pv = psum.tile([D + ONE, 512], f32, tag="pv", bufs=4,
                           name=f"pv{last}_{h2}_{ic}")
            unit_pv[last][(h2, ic)] = pv
            for jk in range(JK):
                nc.tensor.matmul(
                    pv,
                    va_tiles[jk][:, 2 * p_last + h2, :],
                    pt_tiles[(jk, h2)][:, ts(ic, 512)],
                    start=(jk == 0), stop=(jk == JK - 1),
                )
            emit_norm_set(last, h2, ic, dve_mult=True)
            if idx == 1:
                # ic=0 columns of the last obuf are final: overlap the first
                # half of the B projection with the remaining PV chains
                emit_proj(1, its=range(0, 4))
        emit_proj(1, its=range(4, 8))

    if not nc.is_finalized():
        nc.finalize()
    return nc


def get_nc():
    if "nc" not in _CACHE:
        _CACHE["nc"] = _build_nc()
    return _CACHE["nc"]


def make_in_maps(x, w_qkv, b_qkv, w_proj, b_proj):
    x = np.asarray(x)
    w_qkv = np.asarray(w_qkv)
    b_qkv = np.asarray(b_qkv, dtype=np.float32)
    w_proj = np.asarray(w_proj)
    b_proj = np.asarray(b_proj, dtype=np.float32)

    xbts = [np.ascontiguousarray(x[b].T).astype(BF16) for b in range(4)]

    in_maps = []
    for core in range(8):
        b, hs = divmod(core, 2)
        lo = hs * 512
        wq = np.ascontiguousarray(w_qkv[:, lo:lo + 512]).astype(BF16)
        wk = np.ascontiguousarray(w_qkv[:, C + lo:C + lo + 512]).astype(BF16)
        wv = np.ascontiguousarray(w_qkv[:, 2 * C + lo:2 * C + lo + 512]).astype(BF16)
        wp = np.ascontiguousarray(w_proj[lo:lo + 512, :]).astype(BF16)
        bq = np.ascontiguousarray(b_qkv[lo:lo + 512])
        bk = np.ascontiguousarray(b_qkv[C + lo:C + lo + 512])
        bv = np.ascontiguousarray(b_qkv[2 * C + lo:2 * C + lo + 512])
        in_maps.append(dict(xbt=xbts[b], wq=wq, wk=wk, wv=wv, wp=wp,
                            bq=bq, bk=bk, bv=bv))
    return in_maps


def run(x, w_qkv, b_qkv, w_proj, b_proj, trace=False, **kwargs):
    from concourse.bass_utils import run_bass_kernel_spmd
    nc = get_nc()
    in_maps = make_in_maps(x, w_qkv, b_qkv, w_proj, b_proj)
    res = run_bass_kernel_spmd(nc, in_maps, core_ids=list(range(8)),
                               trace=trace, **kwargs)
    B = 4
    bp = np.asarray(b_proj, dtype=np.float32)
    out = np.empty((B, T, C), np.float32)
    for b in range(B):
        out[b] = (res.results[2 * b]["out"].astype(np.float32)
                  + res.results[2 * b + 1]["out"].astype(np.float32)
                  + bp)
    return out, res


def kernel(x, w_qkv, b_qkv, w_proj, b_proj):
    out, _ = run(x, w_qkv, b_qkv, w_proj, b_proj, trace=False)
    return out
